# revision 7
# baseline (speedup 1.0000x reference)
"""Cross-attention kernel for Trainium2, sharded over 8 NeuronCores.

Problem (hardcoded): B=2, N=M=2048, query/context dim 1024, 8 heads x 64.
Sharding: core c -> (batch b=c//4, head-pair hp=c%4). Each core projects
q/k/v for its 2 heads (column-parallel), runs attention for those heads,
and computes a partial output projection (row-parallel over Wo). The host
sums the 4 partials per batch (bf16) and adds the bias.

The ScalarE exp stream (64 x [128,1024] ACTIVATEs ~ 73us) is the hard
floor; the schedule keeps it saturated from ~8us on:
  - weights land first on the HWDGE FIFO, then ctx/x windows in need order
  - warmup matmuls release the HAM clock gate before real work
  - attention is processed in 4-key-tile chunks, window-major across ALL
    four query windows; per-chunk PSUM accumulators are merged into SBUF
    f32 accumulators by the DVE, so PSUM never holds long-lived state and
    chunks from any query window can interleave
  - the inner loop is software-pipelined (attnV lags sim/exp by 2 tiles)
    so the in-order PE queue never head-of-line blocks the exp stream
  - k/q/v projections are spliced between chunks as PE filler
  - v projection computed directly transposed (ctx chunk as stationary)
  - v3 layout [dims | ones] puts S at accumulator row 64 -> only one
    SBUF->SBUF lane-shift DMA per query window (head B)
  - output written bf16, batched DMAs
"""

import numpy as np
import ml_dtypes

B = 2
N = 2048  # query tokens per batch
M = 2048  # context tokens per batch
D = 1024  # query/context feature dim
HEADS = 8
DH = 64
INNER = 512
SCALE = DH**-0.5
P = 128
TW = 512  # token window
NKC = D // P  # contraction chunks for projections (8)
NW = M // TW  # context/query windows (4)
NT = TW // P  # key tiles per window (4)

_STATE = {}


def _build_nc():
    import concourse.bacc as bacc
    import concourse.tile as tile
    import concourse.mybir as mybir
    from concourse.masks import make_identity

    dt = mybir.dt
    bf16 = dt.bfloat16
    f32 = dt.float32

    nc = bacc.Bacc("TRN2", target_bir_lowering=False, debug=False)

    xT = nc.dram_tensor("xT", [NW, P, NKC, TW], bf16, kind="ExternalInput").ap()
    ctxT = nc.dram_tensor("ctxT", [NW, P, NKC, TW], bf16, kind="ExternalInput").ap()
    wq = nc.dram_tensor("wq", [P, NKC, P], bf16, kind="ExternalInput").ap()
    wk = nc.dram_tensor("wk", [P, NKC, P], bf16, kind="ExternalInput").ap()
    wv = nc.dram_tensor("wv", [P, NKC, P], bf16, kind="ExternalInput").ap()
    wo = nc.dram_tensor("wo", [P, 2, 512], bf16, kind="ExternalInput").ap()
    # output blocks: row r = blk*128+p, col = fc*512+c
    outp = nc.dram_tensor("outp", [16, P, 2, 512], bf16, kind="ExternalOutput").ap()

    with tile.TileContext(nc) as tc:
        with (
            tc.tile_pool(name="const", bufs=1) as constp,
            tc.tile_pool(name="weights", bufs=1) as wpool,
            tc.tile_pool(name="persist", bufs=1) as persist,
            tc.tile_pool(name="attn", bufs=6) as apool,
            tc.tile_pool(name="evict", bufs=4) as epool,
            tc.tile_pool(name="norm", bufs=2) as npool,
            tc.tile_pool(name="stage", bufs=2) as spool,
            tc.tile_pool(name="psum_sim", bufs=2, space="PSUM") as psum_sim,
            tc.tile_pool(name="psum_work", bufs=2, space="PSUM") as psum_work,
        ):
            identity = constp.tile([P, P], bf16)
            make_identity(nc, identity)
            identF = constp.tile([P, P], f32)
            make_identity(nc, identF)
            onesF = constp.tile([P, 64], f32)
            nc.vector.memset(onesF[:], 1.0)
            junk = constp.tile([P, TW], bf16)
            nc.vector.memset(junk[:], 0.0)

            # ---- weights FIRST on the HWDGE FIFO (small, unblock projections) ----
            wk_sb = wpool.tile([P, NKC, P], bf16)
            nc.sync.dma_start(wk_sb[:], wk[:])
            wq_sb = wpool.tile([P, NKC, P], bf16)
            nc.sync.dma_start(wq_sb[:], wq[:])
            wv_sb = wpool.tile([P, NKC, P], bf16)
            nc.sync.dma_start(wv_sb[:], wv[:])
            wo_sb = wpool.tile([P, 2, 512], bf16)
            nc.sync.dma_start(wo_sb[:], wo[:])

            # ---- inputs: one batched DMA per window, in consumption order ----
            ctx_sb = persist.tile([P, NW, NKC, TW], bf16)
            x_sb = persist.tile([P, NW, NKC, TW], bf16)

            def load(dst, src, w):
                nc.sync.dma_start(dst[:, w, :, :], src[w])

            load(ctx_sb, ctxT, 0)
            load(x_sb, xT, 0)
            load(x_sb, xT, 1)
            load(ctx_sb, ctxT, 1)
            load(x_sb, xT, 2)
            load(ctx_sb, ctxT, 2)
            load(x_sb, xT, 3)
            load(ctx_sb, ctxT, 3)

            # ---- HAM warmup: ~4us of junk matmuls while DMAs stream ----
            wu = psum_sim.tile([P, TW], f32, tag="sim")
            for _ in range(10):
                nc.tensor.matmul(wu[:], identity[:], junk[:], start=True, stop=True)

            # per-window persistent k (transposed) and v (natural + ones col)
            # v3 layout per head: [64 dims | ones] -> S lands at acc row 64
            kTw = [persist.tile([P, TW], bf16, name=f"kTw{w}", tag=f"kTw{w}") for w in range(NW)]
            v3w = [persist.tile([P, NT, 130], bf16, name=f"v3w{w}", tag=f"v3w{w}") for w in range(NW)]
            qws = [persist.tile([P, TW], bf16, name=f"qw{w}", tag=f"qw{w}") for w in range(NW)]
            # f32 output accumulators per query window: rows 0-63 = o, 64 = S
            o_sb = [
                persist.tile([65, 2, TW], f32, name=f"osb{iw}", tag=f"osb{iw}")
                for iw in range(NW)
            ]
            for w in range(NW):
                nc.vector.memset(v3w[w][:, :, 64:65], 1.0)
                nc.vector.memset(v3w[w][:, :, 129:130], 1.0)

            def proj_k(w):
                psk = psum_work.tile([P, TW], f32, tag="work")
                for kc in range(NKC):
                    nc.tensor.matmul(
                        psk[:], wk_sb[:, kc, :], ctx_sb[:, w, kc, :],
                        start=(kc == 0), stop=(kc == NKC - 1),
                    )
                nc.vector.tensor_copy(kTw[w][:], psk[:])

            def proj_q(w):
                psq = psum_work.tile([P, TW], f32, tag="work")
                for kc in range(NKC):
                    nc.tensor.matmul(
                        psq[:], wq_sb[:, kc, :], x_sb[:, w, kc, :],
                        start=(kc == 0), stop=(kc == NKC - 1),
                    )
                nc.vector.tensor_copy(qws[w][:], psq[:])

            def proj_v(w):
                # directly transposed: [keys, dims], ctx chunk stationary
                vt = psum_work.tile([P, NT, P], f32, tag="work")
                for t in range(NT):
                    ksl = slice(t * P, (t + 1) * P)
                    for kc in range(NKC):
                        nc.tensor.matmul(
                            vt[:, t, :], ctx_sb[:, w, kc, ksl], wv_sb[:, kc, :],
                            start=(kc == 0), stop=(kc == NKC - 1),
                        )
                nc.vector.tensor_copy(v3w[w][:, :, 0:64], vt[:, :, 0:64])
                nc.vector.tensor_copy(v3w[w][:, :, 65:129], vt[:, :, 64:128])

            # per-chunk psum accumulators, merged to o_sb after each chunk
            chunk_ps = {}
            pending = []

            def SE(iw, w, t):
                jsl = slice(t * P, (t + 1) * P)
                s2 = psum_sim.tile([P, 2 * TW], f32, tag="sim")
                nc.tensor.matmul(
                    s2[:, 0:TW], kTw[w][0:64, jsl], qws[iw][0:64, :],
                    skip_group_check=True,
                )
                nc.tensor.matmul(
                    s2[:, TW:], kTw[w][64:128, jsl], qws[iw][64:128, :],
                    skip_group_check=True,
                )
                a2 = apool.tile([P, 2 * TW], bf16, tag="a")
                nc.scalar.activation(
                    a2[:], s2[:], mybir.ActivationFunctionType.Exp, scale=SCALE
                )
                pending.append((iw, w, t, a2))

            def F():
                iw, w, t, a2 = pending.pop(0)
                if (iw, w) not in chunk_ps:
                    chunk_ps[(iw, w)] = psum_work.tile(
                        [65, 2, TW], f32, name=f"cp{iw}_{w}", tag="work"
                    )
                cp = chunk_ps[(iw, w)]
                nc.tensor.matmul(
                    cp[:, 0, :], v3w[w][:, t, 0:65], a2[:, 0:TW],
                    start=(t == 0), stop=(t == NT - 1), skip_group_check=True,
                )
                nc.tensor.matmul(
                    cp[:, 1, :], v3w[w][:, t, 65:130], a2[:, TW:],
                    start=(t == 0), stop=(t == NT - 1), skip_group_check=True,
                )

            def MERGE(iw, w):
                cp = chunk_ps.pop((iw, w))
                for h in range(2):
                    if w == 0:
                        nc.vector.tensor_copy(o_sb[iw][:, h, :], cp[:, h, :])
                    else:
                        nc.vector.tensor_add(
                            o_sb[iw][:, h, :], o_sb[iw][:, h, :], cp[:, h, :]
                        )

            aos = {}

            def NORM(iw):
                # broadcast 1/S across partitions: rows 0-63 <- S_A, 64-127 <- S_B
                bc_ps = psum_work.tile([P, TW], f32, tag="work")
                nc.tensor.matmul(bc_ps[0:64, :], onesF[64:65, :], o_sb[iw][64:65, 0, :])
                nc.tensor.matmul(bc_ps[64:128, :], onesF[64:65, :], o_sb[iw][64:65, 1, :])
                rbc = npool.tile([P, TW], f32, tag="rbc")
                nc.vector.reciprocal_approx_fast(rbc[:], bc_ps[:])
                # head B lane-shift into rows 64-127 via col-tiled identity MM
                sh_ps = psum_work.tile([P, TW], f32, tag="work")
                nc.tensor.matmul(
                    sh_ps[64:128, :], identF[0:64, 0:64], o_sb[iw][0:64, 1, :]
                )
                ao = npool.tile([P, TW], bf16, tag="ao")
                nc.vector.tensor_mul(ao[0:64, :], o_sb[iw][0:64, 0, :], rbc[0:64, :])
                nc.vector.tensor_mul(ao[64:128, :], sh_ps[64:128, :], rbc[64:128, :])
                aos[iw] = ao

            def OPROJ(iw):
                ao = aos.pop(iw)
                stage = spool.tile([P, 4, 2, 512], bf16, tag="st")
                for it in range(NT):
                    op_ps = psum_work.tile([P, 2, 512], f32, tag="work")
                    for fc in range(2):
                        nc.tensor.matmul(
                            op_ps[:, fc, :], ao[:, it * P : (it + 1) * P],
                            wo_sb[:, fc, :],
                        )
                    nc.vector.tensor_copy(stage[:, it, :, :], op_ps[:])
                for it in range(NT):
                    nc.gpsimd.dma_start(outp[iw * 4 + it], stage[:, it, :, :])

            # ---- schedule: window-major chunks, PE fillers spliced in ----
            # pre-fillers run before the chunk's first sim; mid-fillers run
            # after MERGE of the previous chunk (so they see its results and
            # a freed psum_work slot) but before the chunk's 3rd key tile,
            # whose flush is the first attnV that may need a new v3 window.
            chunks = [
                (0, 0), (1, 0), (0, 1), (1, 1),
                (2, 0), (3, 0), (2, 1), (3, 1),
                (0, 2), (1, 2), (2, 2), (3, 2),
                (0, 3), (1, 3), (2, 3), (3, 3),
            ]
            pre = {
                0: [lambda: proj_k(0), lambda: proj_q(0)],
                1: [lambda: proj_q(1)],
                2: [lambda: proj_k(1)],
                4: [lambda: proj_q(2)],
                5: [lambda: proj_q(3)],
                7: [lambda: proj_k(2)],
                11: [lambda: proj_k(3)],
            }
            mid = {
                2: [lambda: proj_v(1)],
                8: [lambda: proj_v(2)],
                12: [lambda: proj_v(3)],
                13: [lambda: NORM(0)],
                14: [lambda: NORM(1), lambda: OPROJ(0)],
                15: [lambda: NORM(2), lambda: OPROJ(1)],
            }
            for ci, (iw, w) in enumerate(chunks):
                for f in pre.get(ci, []):
                    f()
                SE(iw, w, 0)
                if ci == 0:
                    SE(iw, w, 1)
                    proj_v(0)
                    F()
                    F()
                    SE(iw, w, 2)
                    SE(iw, w, 3)
                else:
                    F()
                    SE(iw, w, 1)
                    F()
                    MERGE(*chunks[ci - 1])
                    for f in mid.get(ci, []):
                        f()
                    SE(iw, w, 2)
                    F()
                    SE(iw, w, 3)
                    F()
            F()
            F()
            MERGE(*chunks[15])
            OPROJ(2)
            NORM(3)
            OPROJ(3)

    nc.compile()
    return nc


def _get_nc():
    if "nc" not in _STATE:
        _STATE["nc"] = _build_nc()
    return _STATE["nc"]


def _make_in_maps(x, context, Wq, Wk, Wv, Wo):
    bf = ml_dtypes.bfloat16

    def wslice(W, hp):
        # [1024, 128] -> [p, kc, m] with k = kc*128 + p
        s = W[:, hp * P : (hp + 1) * P]
        return np.ascontiguousarray(
            s.reshape(NKC, P, P).transpose(1, 0, 2)
        ).astype(bf)

    def tlay(a):
        # [tokens, D] -> [w, p, kc, t] with d = kc*128 + p
        return np.ascontiguousarray(
            a.T.reshape(NKC, P, NW, TW).transpose(2, 1, 0, 3)
        ).astype(bf)

    xTs = [tlay(x[b]) for b in range(B)]
    cTs = [tlay(context[b]) for b in range(B)]
    in_maps = []
    for c in range(8):
        b, hp = c // 4, c % 4
        in_maps.append(
            {
                "xT": xTs[b],
                "ctxT": cTs[b],
                "wq": wslice(Wq, hp),
                "wk": wslice(Wk, hp),
                "wv": wslice(Wv, hp),
                "wo": np.ascontiguousarray(
                    Wo[hp * P : (hp + 1) * P, :].reshape(P, 2, 512)
                ).astype(bf),
            }
        )
    return in_maps


def kernel(x, context, Wq, Wk, Wv, Wo, bo, _spmd_kwargs=None):
    from concourse.bass_utils import run_bass_kernel_spmd

    nc = _get_nc()
    in_maps = _make_in_maps(x, context, Wq, Wk, Wv, Wo)
    res = run_bass_kernel_spmd(
        nc, in_maps, core_ids=list(range(8)), **(_spmd_kwargs or {})
    )
    _STATE["last_result"] = res
    outs = [
        np.asarray(r["outp"]).astype(np.float32).reshape(N, D) for r in res.results
    ]
    out = np.empty((B, N, D), np.float32)
    for b in range(B):
        out[b] = outs[4 * b] + outs[4 * b + 1] + outs[4 * b + 2] + outs[4 * b + 3]
        out[b] += bo.astype(np.float32)
    return out


# revision 8
# speedup vs baseline: 1.0083x; 1.0083x over previous
"""Cross-attention kernel for Trainium2, sharded over 8 NeuronCores.

Problem (hardcoded): B=2, N=M=2048, query/context dim 1024, 8 heads x 64.
Sharding: core c -> (batch b=c//4, head-pair hp=c%4). Each core projects
q/k/v for its 2 heads (column-parallel), runs attention for those heads,
and computes a partial output projection (row-parallel over Wo). The host
sums the 4 partials per batch (bf16) and adds the bias.

The ScalarE exp stream (64 x [128,1024] ACTIVATEs ~ 73us) is the hard
floor; the schedule keeps it saturated from ~8us on:
  - weights land first on the HWDGE FIFO, then ctx/x windows in need order
  - warmup matmuls release the HAM clock gate before real work
  - attention is processed in 4-key-tile chunks, window-major across ALL
    four query windows; per-chunk PSUM accumulators are merged into SBUF
    f32 accumulators by the DVE, so PSUM never holds long-lived state and
    chunks from any query window can interleave
  - the inner loop is software-pipelined (attnV lags sim/exp by 2 tiles)
    so the in-order PE queue never head-of-line blocks the exp stream
  - k/q/v projections are spliced between chunks as PE filler
  - v projection computed directly transposed (ctx chunk as stationary)
  - v3 layout [dims | ones] puts S at accumulator row 64 -> only one
    SBUF->SBUF lane-shift DMA per query window (head B)
  - output written bf16, batched DMAs
"""

import numpy as np
import ml_dtypes

B = 2
N = 2048  # query tokens per batch
M = 2048  # context tokens per batch
D = 1024  # query/context feature dim
HEADS = 8
DH = 64
INNER = 512
SCALE = DH**-0.5
P = 128
TW = 512  # token window
NKC = D // P  # contraction chunks for projections (8)
NW = M // TW  # context/query windows (4)
NT = TW // P  # key tiles per window (4)

_STATE = {}


def _build_nc():
    import concourse.bacc as bacc
    import concourse.tile as tile
    import concourse.mybir as mybir
    from concourse.masks import make_identity

    dt = mybir.dt
    bf16 = dt.bfloat16
    f32 = dt.float32

    nc = bacc.Bacc("TRN2", target_bir_lowering=False, debug=False)

    xT = nc.dram_tensor("xT", [NW, P, NKC, TW], bf16, kind="ExternalInput").ap()
    ctxT = nc.dram_tensor("ctxT", [NW, P, NKC, TW], bf16, kind="ExternalInput").ap()
    wq = nc.dram_tensor("wq", [P, NKC, P], bf16, kind="ExternalInput").ap()
    wk = nc.dram_tensor("wk", [P, NKC, P], bf16, kind="ExternalInput").ap()
    wv = nc.dram_tensor("wv", [P, NKC, P], bf16, kind="ExternalInput").ap()
    wo = nc.dram_tensor("wo", [P, 2, 512], bf16, kind="ExternalInput").ap()
    # output blocks: row r = blk*128+p, col = fc*512+c
    outp = nc.dram_tensor("outp", [16, P, 2, 512], bf16, kind="ExternalOutput").ap()

    with tile.TileContext(nc) as tc:
        with (
            tc.tile_pool(name="const", bufs=1) as constp,
            tc.tile_pool(name="weights", bufs=1) as wpool,
            tc.tile_pool(name="persist", bufs=1) as persist,
            tc.tile_pool(name="attn", bufs=6) as apool,
            tc.tile_pool(name="evict", bufs=4) as epool,
            tc.tile_pool(name="norm", bufs=2) as npool,
            tc.tile_pool(name="stage", bufs=2) as spool,
            tc.tile_pool(name="psum_sim", bufs=2, space="PSUM") as psum_sim,
            tc.tile_pool(name="psum_work", bufs=2, space="PSUM") as psum_work,
        ):
            identity = constp.tile([P, P], bf16)
            make_identity(nc, identity)
            identF = constp.tile([P, P], f32)
            make_identity(nc, identF)
            onesF = constp.tile([P, 64], f32)
            nc.vector.memset(onesF[:], 1.0)
            junk = constp.tile([P, TW], bf16)
            nc.vector.memset(junk[:], 0.0)

            # ---- DMAs ordered by first consumer: wk, ctx0, wq, x0, ... ----
            ctx_sb = persist.tile([P, NW, NKC, TW], bf16)
            x_sb = persist.tile([P, NW, NKC, TW], bf16)

            def load(dst, src, w):
                nc.sync.dma_start(dst[:, w, :, :], src[w])

            wk_sb = wpool.tile([P, NKC, P], bf16)
            nc.sync.dma_start(wk_sb[:], wk[:])
            load(ctx_sb, ctxT, 0)
            wq_sb = wpool.tile([P, NKC, P], bf16)
            nc.sync.dma_start(wq_sb[:], wq[:])
            load(x_sb, xT, 0)
            wv_sb = wpool.tile([P, NKC, P], bf16)
            nc.sync.dma_start(wv_sb[:], wv[:])
            wo_sb = wpool.tile([P, 2, 512], bf16)
            nc.sync.dma_start(wo_sb[:], wo[:])
            load(x_sb, xT, 1)
            load(ctx_sb, ctxT, 1)
            load(x_sb, xT, 2)
            load(ctx_sb, ctxT, 2)
            load(x_sb, xT, 3)
            load(ctx_sb, ctxT, 3)

            # ---- HAM warmup: ~4us of junk matmuls while DMAs stream ----
            wu = psum_sim.tile([P, TW], f32, tag="sim")
            for _ in range(12):
                nc.tensor.matmul(wu[:], identity[:], junk[:], start=True, stop=True)

            # per-window persistent k (transposed) and v (natural + ones col)
            # v3 layout per head: [64 dims | ones] -> S lands at acc row 64
            kTw = [persist.tile([P, TW], bf16, name=f"kTw{w}", tag=f"kTw{w}") for w in range(NW)]
            v3w = [persist.tile([P, NT, 130], bf16, name=f"v3w{w}", tag=f"v3w{w}") for w in range(NW)]
            qws = [persist.tile([P, TW], bf16, name=f"qw{w}", tag=f"qw{w}") for w in range(NW)]
            # f32 output accumulators per query window: rows 0-63 = o, 64 = S
            o_sb = [
                persist.tile([65, 2, TW], f32, name=f"osb{iw}", tag=f"osb{iw}")
                for iw in range(NW)
            ]
            for w in range(NW):
                nc.vector.memset(v3w[w][:, :, 64:65], 1.0)
                nc.vector.memset(v3w[w][:, :, 129:130], 1.0)

            def proj_k(w):
                psk = psum_work.tile([P, TW], f32, tag="work")
                for kc in range(NKC):
                    nc.tensor.matmul(
                        psk[:], wk_sb[:, kc, :], ctx_sb[:, w, kc, :],
                        start=(kc == 0), stop=(kc == NKC - 1),
                    )
                nc.vector.tensor_copy(kTw[w][:], psk[:])

            def proj_q(w):
                psq = psum_work.tile([P, TW], f32, tag="work")
                for kc in range(NKC):
                    nc.tensor.matmul(
                        psq[:], wq_sb[:, kc, :], x_sb[:, w, kc, :],
                        start=(kc == 0), stop=(kc == NKC - 1),
                    )
                nc.vector.tensor_copy(qws[w][:], psq[:])

            def proj_v(w):
                # directly transposed: [keys, dims], ctx chunk stationary
                vt = psum_work.tile([P, NT, P], f32, tag="work")
                for t in range(NT):
                    ksl = slice(t * P, (t + 1) * P)
                    for kc in range(NKC):
                        nc.tensor.matmul(
                            vt[:, t, :], ctx_sb[:, w, kc, ksl], wv_sb[:, kc, :],
                            start=(kc == 0), stop=(kc == NKC - 1),
                        )
                nc.vector.tensor_copy(v3w[w][:, :, 0:64], vt[:, :, 0:64])
                nc.vector.tensor_copy(v3w[w][:, :, 65:129], vt[:, :, 64:128])

            # per-chunk psum accumulators, merged to o_sb after each chunk
            chunk_ps = {}
            pending = []

            def SE(iw, w, t):
                jsl = slice(t * P, (t + 1) * P)
                s2 = psum_sim.tile([P, 2 * TW], f32, tag="sim")
                nc.tensor.matmul(
                    s2[:, 0:TW], kTw[w][0:64, jsl], qws[iw][0:64, :],
                    skip_group_check=True,
                )
                nc.tensor.matmul(
                    s2[:, TW:], kTw[w][64:128, jsl], qws[iw][64:128, :],
                    skip_group_check=True,
                )
                a2 = apool.tile([P, 2 * TW], bf16, tag="a")
                nc.scalar.activation(
                    a2[:], s2[:], mybir.ActivationFunctionType.Exp, scale=SCALE
                )
                pending.append((iw, w, t, a2))

            def F():
                iw, w, t, a2 = pending.pop(0)
                if (iw, w) not in chunk_ps:
                    chunk_ps[(iw, w)] = psum_work.tile(
                        [65, 2, TW], f32, name=f"cp{iw}_{w}", tag="work"
                    )
                cp = chunk_ps[(iw, w)]
                nc.tensor.matmul(
                    cp[:, 0, :], v3w[w][:, t, 0:65], a2[:, 0:TW],
                    start=(t == 0), stop=(t == NT - 1), skip_group_check=True,
                )
                nc.tensor.matmul(
                    cp[:, 1, :], v3w[w][:, t, 65:130], a2[:, TW:],
                    start=(t == 0), stop=(t == NT - 1), skip_group_check=True,
                )

            def MERGE(iw, w):
                cp = chunk_ps.pop((iw, w))
                for h in range(2):
                    if w == 0:
                        nc.vector.tensor_copy(o_sb[iw][:, h, :], cp[:, h, :])
                    else:
                        nc.vector.tensor_add(
                            o_sb[iw][:, h, :], o_sb[iw][:, h, :], cp[:, h, :]
                        )

            aos = {}

            def NORM(iw):
                # broadcast 1/S across partitions: rows 0-63 <- S_A, 64-127 <- S_B
                bc_ps = psum_work.tile([P, TW], f32, tag="work")
                nc.tensor.matmul(bc_ps[0:64, :], onesF[64:65, :], o_sb[iw][64:65, 0, :])
                nc.tensor.matmul(bc_ps[64:128, :], onesF[64:65, :], o_sb[iw][64:65, 1, :])
                rbc = npool.tile([P, TW], f32, tag="rbc")
                nc.vector.reciprocal_approx_fast(rbc[:], bc_ps[:])
                # head B lane-shift into rows 64-127 via col-tiled identity MM
                sh_ps = psum_work.tile([P, TW], f32, tag="work")
                nc.tensor.matmul(
                    sh_ps[64:128, :], identF[0:64, 0:64], o_sb[iw][0:64, 1, :]
                )
                ao = npool.tile([P, TW], bf16, tag="ao")
                nc.vector.tensor_mul(ao[0:64, :], o_sb[iw][0:64, 0, :], rbc[0:64, :])
                nc.vector.tensor_mul(ao[64:128, :], sh_ps[64:128, :], rbc[64:128, :])
                aos[iw] = ao

            stages = {}

            def OPROJ_it(iw, it, evict="v"):
                if iw not in stages:
                    stages[iw] = spool.tile(
                        [P, 4, 2, 512], bf16, name=f"st{iw}", tag="st"
                    )
                stage = stages[iw]
                ao = aos[iw]
                op_ps = psum_work.tile([P, 2, 512], f32, tag="work")
                for fc in range(2):
                    nc.tensor.matmul(
                        op_ps[:, fc, :], ao[:, it * P : (it + 1) * P],
                        wo_sb[:, fc, :],
                    )
                if evict == "v":
                    nc.vector.tensor_copy(stage[:, it, :, :], op_ps[:])
                else:
                    nc.scalar.copy(stage[:, it, :, :], op_ps[:])
                nc.gpsimd.dma_start(outp[iw * 4 + it], stage[:, it, :, :])

            # ---- schedule: window-major chunks, PE fillers spliced in ----
            # pre-fillers run before the chunk's first sim; mid-fillers run
            # after MERGE of the previous chunk (so they see its results and
            # a freed psum_work slot) but before the chunk's 3rd key tile,
            # whose flush is the first attnV that may need a new v3 window.
            chunks = [
                (0, 0), (1, 0), (0, 1), (1, 1),
                (2, 0), (3, 0), (2, 1), (3, 1),
                (0, 2), (1, 2), (2, 2), (3, 2),
                (0, 3), (1, 3), (2, 3), (3, 3),
            ]
            pre = {
                0: [lambda: proj_k(0), lambda: proj_q(0)],
                1: [lambda: proj_q(1)],
                2: [lambda: proj_k(1)],
                4: [lambda: proj_q(2)],
                5: [lambda: proj_q(3)],
                7: [lambda: proj_k(2)],
                11: [lambda: proj_k(3)],
            }
            mid = {
                2: [lambda: proj_v(1)],
                8: [lambda: proj_v(2)],
                12: [lambda: proj_v(3)],
                13: [lambda: NORM(0)],
                14: [lambda: NORM(1)],
                15: [lambda: NORM(2)],
            }
            post2 = {
                14: [lambda: OPROJ_it(0, 0), lambda: OPROJ_it(0, 1)],
                15: [lambda: OPROJ_it(1, 0), lambda: OPROJ_it(1, 1)],
            }
            post3 = {
                14: [lambda: OPROJ_it(0, 2), lambda: OPROJ_it(0, 3)],
                15: [lambda: OPROJ_it(1, 2), lambda: OPROJ_it(1, 3)],
            }
            for ci, (iw, w) in enumerate(chunks):
                for f in pre.get(ci, []):
                    f()
                SE(iw, w, 0)
                if ci == 0:
                    SE(iw, w, 1)
                    proj_v(0)
                    F()
                    F()
                    SE(iw, w, 2)
                    SE(iw, w, 3)
                else:
                    F()
                    SE(iw, w, 1)
                    F()
                    MERGE(*chunks[ci - 1])
                    for f in mid.get(ci, []):
                        f()
                    SE(iw, w, 2)
                    F()
                    for f in post2.get(ci, []):
                        f()
                    SE(iw, w, 3)
                    F()
                    for f in post3.get(ci, []):
                        f()
            F()
            F()
            MERGE(*chunks[15])
            for it in range(NT):
                OPROJ_it(2, it, evict="s" if it % 2 == 0 else "v")
            NORM(3)
            for it in range(NT):
                OPROJ_it(3, it, evict="s" if it % 2 == 0 else "v")

    nc.compile()
    return nc


def _get_nc():
    if "nc" not in _STATE:
        _STATE["nc"] = _build_nc()
    return _STATE["nc"]


def _make_in_maps(x, context, Wq, Wk, Wv, Wo):
    bf = ml_dtypes.bfloat16

    def wslice(W, hp):
        # [1024, 128] -> [p, kc, m] with k = kc*128 + p
        s = W[:, hp * P : (hp + 1) * P]
        return np.ascontiguousarray(
            s.reshape(NKC, P, P).transpose(1, 0, 2)
        ).astype(bf)

    def tlay(a):
        # [tokens, D] -> [w, p, kc, t] with d = kc*128 + p
        return np.ascontiguousarray(
            a.T.reshape(NKC, P, NW, TW).transpose(2, 1, 0, 3)
        ).astype(bf)

    xTs = [tlay(x[b]) for b in range(B)]
    cTs = [tlay(context[b]) for b in range(B)]
    in_maps = []
    for c in range(8):
        b, hp = c // 4, c % 4
        in_maps.append(
            {
                "xT": xTs[b],
                "ctxT": cTs[b],
                "wq": wslice(Wq, hp),
                "wk": wslice(Wk, hp),
                "wv": wslice(Wv, hp),
                "wo": np.ascontiguousarray(
                    Wo[hp * P : (hp + 1) * P, :].reshape(P, 2, 512)
                ).astype(bf),
            }
        )
    return in_maps


def kernel(x, context, Wq, Wk, Wv, Wo, bo, _spmd_kwargs=None):
    from concourse.bass_utils import run_bass_kernel_spmd

    nc = _get_nc()
    in_maps = _make_in_maps(x, context, Wq, Wk, Wv, Wo)
    res = run_bass_kernel_spmd(
        nc, in_maps, core_ids=list(range(8)), **(_spmd_kwargs or {})
    )
    _STATE["last_result"] = res
    outs = [
        np.asarray(r["outp"]).astype(np.float32).reshape(N, D) for r in res.results
    ]
    out = np.empty((B, N, D), np.float32)
    for b in range(B):
        out[b] = outs[4 * b] + outs[4 * b + 1] + outs[4 * b + 2] + outs[4 * b + 3]
        out[b] += bo.astype(np.float32)
    return out
